# revision 8
# baseline (speedup 1.0000x reference)
"""Trainium2 Bass kernel for the diagonal-Radon problem.

Math: the reference computes a full parallel-beam forward projection
sino[b,c,d,a] and keeps only the diagonal d==c.  So for channel j we only
need the line integral at detector offset (j-63.5) of image X[b,j]:

    out[b,j,a] = sum_t bilinear(X[b,j], u, v)
    u = 63.5 + (j-63.5)cos(th_a) - (t-63.5)sin(th_a)
    v = 63.5 + (j-63.5)sin(th_a) + (t-63.5)cos(th_a)

Device strategy (per NeuronCore, 16 channels per core, 2 passes of 8):
  - SBUF partitions = 8 GPSIMD groups (one channel each) x 16 lanes
    (2 column-shifted interleaved image copies x 8 batches).  The image is
    stored row-interleaved: block (pb,qb) of lane cs holds
    [X[pb-1, qb-1+cs], X[pb, qb-1+cs]], so ONE block index per (angle,t)
    sample fetches all 4 bilinear corners across the lanes, for all 8
    batches at once, via the extended-ISA `ap_gather` GPSIMD op (all 16
    lanes of a group share one index stream).
  - Bilinear weights (with the reference's exact validity masking and
    boundary remaps) are precomputed on the host from `angles`, replicated
    over batch lanes, DMA'd in; DVE does gathered*weight and a segmented
    t-reduction per angle.
  - Host sums the (cs, r) corner partials and reassembles [B,C,1,A].
"""

import numpy as np

N = 128
B = 8
C = 128
A = 180
C0 = np.float32(63.5)
NBLK = N * N         # 16384 blocks of 2 elements -> 32768 f32 per lane
SPP = A * N          # samples per channel = 23040
KA = 6               # angles per chunk (768 idx/gather call: ~25ns/idx vs
NCH = A // KA        # 30 chunks        ~31ns/idx at 1536 — measured)
NCORES = 8
JPC = 16             # channels per core
NPASS = 2
JPP = 8              # channels per pass

LAST_RESULT = None

_prog_cache = {}

# Pair symmetry: channel c (detector d) and 127-c (detector -d) trace
# point-symmetric rays.  out(127-c,a) = sum_t bilin(flip2(X[127-c]), P_c(t))
# with channel c's exact indices AND weights (masks mirror exactly).
# bf16 2x2-interleaved blocks (d=4) put the column pair inside the block,
# freeing lanes for (m=pair-member, b=batch): ONE pass, half the indices.


def _build_program(reps=1):
    import concourse.bacc as bacc
    import concourse.mybir as mybir
    import concourse.tile as tile

    nc = bacc.Bacc("TRN2", target_bir_lowering=False, debug=False,
                   num_devices=NCORES)
    f32 = mybir.dt.float32
    bf16 = mybir.dt.bfloat16
    i16 = mybir.dt.int16

    xs_in = [nc.dram_tensor("xs0", [128, 4 * NBLK], bf16,
                            kind="ExternalInput").ap()]
    idx_in = [nc.dram_tensor("idx0", [128, SPP // 16], i16,
                             kind="ExternalInput").ap()]
    wq_in = [nc.dram_tensor("wq0", [128, SPP * 4], bf16,
                            kind="ExternalInput").ap()]
    res_out = [nc.dram_tensor("res0", [128, A], f32,
                              kind="ExternalOutput").ap()]

    ns = KA * N          # samples per chunk (per base channel)
    with tile.TileContext(nc) as tc:
        with tc.tile_pool(name="xsp", bufs=1) as xsp, \
             tc.tile_pool(name="idxp", bufs=1) as idxp, \
             tc.tile_pool(name="wqp", bufs=2) as wqp, \
             tc.tile_pool(name="gp", bufs=2) as gp, \
             tc.tile_pool(name="resp", bufs=1) as resp:
          for _rep in range(reps):
            xs_t = xsp.tile([128, 4 * NBLK], bf16)
            nc.sync.dma_start(xs_t[:], xs_in[0])
            idx_t = idxp.tile([128, SPP // 16], i16)
            nc.sync.dma_start(idx_t[:], idx_in[0])
            res_t = resp.tile([128, A], f32)
            for k in range(NCH):
                wq_t = wqp.tile([128, ns * 4], bf16)
                nc.sync.dma_start(
                    wq_t[:], wq_in[0][:, k * ns * 4:(k + 1) * ns * 4])
                g_t = gp.tile([128, ns * 4], bf16)
                nc.gpsimd.ap_gather(
                    out_ap=g_t[:].rearrange("p (n d) -> p n d", d=4),
                    in_ap=xs_t[:].rearrange("p (n d) -> p n d", d=4),
                    idxs_ap=idx_t[:, k * (ns // 16):(k + 1) * (ns // 16)],
                    channels=128,
                    num_elems=NBLK,
                    d=4,
                    num_idxs=ns,
                )
                nc.vector.tensor_mul(g_t[:], g_t[:], wq_t[:])
                nc.vector.tensor_reduce(
                    res_t[:, k * KA:(k + 1) * KA],
                    g_t[:].rearrange("p (a w) -> p a w", w=4 * N),
                    axis=mybir.AxisListType.X,
                    op=mybir.AluOpType.add,
                    opt_input=False,
                )
            nc.sync.dma_start(res_out[0], res_t[:])
    nc.compile()
    return nc


def _host_tables(angles):
    """Per-(j,a,t) block indices and per-(cs,r)-corner masked bilinear
    weights.  Mirrors the reference's fp32 arithmetic order.

    Returns idx [C,A,N] int16 and W [2cs,2r,C,A,N] f32 where the (cs,r)
    corner maps to image point (pb-1+r, qb-1+cs)."""
    ang = np.asarray(angles, dtype=np.float32)
    cosv = np.cos(ang).astype(np.float32)
    sinv = np.sin(ang).astype(np.float32)
    jj = (np.arange(C, dtype=np.float32) - C0)[:, None, None]
    tt = (np.arange(N, dtype=np.float32) - C0)[None, None, :]
    cosb = cosv[None, :, None]
    sinb = sinv[None, :, None]

    u = (C0 + jj * cosb) - tt * sinb
    v = (C0 + jj * sinb) + tt * cosb
    u0 = np.floor(u)
    v0 = np.floor(v)
    wu = u - u0
    wv = v - v0
    p0 = u0.astype(np.int32)
    q0 = v0.astype(np.int32)

    pb = np.clip(p0 + 1, 0, N - 1)
    qb = np.clip(q0 + 1, 0, N - 1)
    idx = (pb * N + qb).astype(np.int16)

    one = np.float32(1.0)
    zero = np.float32(0.0)
    w = np.empty((2, 2, C, A, N), dtype=np.float32)
    for cs in range(2):
        col = qb - 1 + cs
        wcol = np.where(col == q0, one - wv, np.where(col == q0 + 1, wv, zero))
        colok = ((col >= 0) & (col < N)).astype(np.float32)
        # note: col==q0+1 only "valid" in reference if q0+1 < N, which colok
        # enforces; col==q0 needs q0 >= 0, also colok.
        wc = wcol * colok
        for r in range(2):
            row = pb - 1 + r
            wrow = np.where(row == p0, one - wu,
                            np.where(row == p0 + 1, wu, zero))
            rowok = ((row >= 0) & (row < N)).astype(np.float32)
            w[cs, r] = (wrow * rowok) * wc
    return idx, w


def _bf16(a):
    import ml_dtypes
    return a.astype(ml_dtypes.bfloat16)


def _core_inputs(X, idx, w, core):
    """Build the per-core input map for chip-core `core`.

    Core handles 8 pairs (base c = 8*core+g, partner 127-c).  Partition
    p = g*16 + m*8 + b; lane m=0 holds X[base], m=1 holds X[partner]
    flipped in both spatial axes.  Block e-order: e = r*2 + cs, value
    pad[pb-1+r, qb-1+cs]; indices and weights are the BASE channel's.
    """
    bases = 8 * core + np.arange(8)
    ins = {}

    xs = np.zeros((8, 2, 8, 4 * NBLK), dtype=np.float32)   # [g,m,b,flat]
    pad = np.zeros((B, N + 2, N + 2), dtype=np.float32)
    for g in range(8):
        for m in range(2):
            if m == 0:
                img = X[:, bases[g]]
            else:
                img = X[:, 127 - bases[g]][:, ::-1, ::-1]
            pad[:, 1:N + 1, 1:N + 1] = img
            # flat[blk*4 + r*2 + cs] = pad[pb+r, qb+cs]  (pad idx = coord+1)
            blk = np.stack([pad[:, r:r + N, cs:cs + N]
                            for r in range(2) for cs in range(2)], axis=-1)
            xs[g, m] = blk.reshape(B, 4 * NBLK)
    ins["xs0"] = _bf16(xs.reshape(128, 4 * NBLK))

    idxw = np.empty((8, 16, SPP // 16), dtype=np.int16)
    for g in range(8):
        stream = idx[bases[g]].reshape(SPP)                # a-major
        idxw[g] = stream.reshape(SPP // 16, 16).T
    ins["idx0"] = idxw.reshape(128, SPP // 16)

    # wq[p, (a,t,e)] with e=(r,cs); identical for all 16 (m,b) lanes of g
    sub = w[:, :, bases]                                   # [2cs,2r,8g,A,N]
    arr = sub.transpose(2, 3, 4, 1, 0)                     # [g,A,N,r,cs]
    arr = arr.reshape(8, 1, SPP * 4)
    arr = np.broadcast_to(arr, (8, 16, SPP * 4))
    ins["wq0"] = _bf16(np.ascontiguousarray(arr).reshape(128, SPP * 4))
    return ins


def kernel(X, angles):
    global LAST_RESULT
    import os
    # No NTFF/axon profiling hook in this environment; make sure a stray
    # BASS_TRACE=1 can't route us into the missing antenv.axon_hooks import.
    os.environ["BASS_NEVER_TRACE"] = "1"
    from concourse.bass_utils import run_bass_kernel_spmd

    X = np.ascontiguousarray(np.asarray(X, dtype=np.float32))
    if "nc" not in _prog_cache:
        _prog_cache["nc"] = _build_program()
    nc = _prog_cache["nc"]

    akey = np.asarray(angles, dtype=np.float32).tobytes()
    if _prog_cache.get("akey") != akey:
        _prog_cache["tables"] = _host_tables(angles)
        _prog_cache["akey"] = akey
    idx, w = _prog_cache["tables"]
    in_maps = [_core_inputs(X, idx, w, c) for c in range(NCORES)]
    _prog_cache["in_maps"] = in_maps

    result = run_bass_kernel_spmd(
        nc, in_maps, core_ids=list(range(NCORES)), trace=False)
    LAST_RESULT = result

    out = np.zeros((B, C, 1, A), dtype=np.float32)
    for c in range(NCORES):
        res = result.results[c]["res0"].reshape(8, 2, 8, A)   # [g,m,b,A]
        bases = 8 * c + np.arange(8)
        out[:, bases, 0, :] = res[:, 0].transpose(1, 0, 2)
        out[:, 127 - bases, 0, :] = res[:, 1].transpose(1, 0, 2)
    return out


# ---------------------------------------------------------------------------
# Timing support (no NTFF profiling hook in this environment): slope method.
# ---------------------------------------------------------------------------

def _make_sharded_callable(nc):
    import jax
    from jax.sharding import Mesh, PartitionSpec, NamedSharding
    from jax.experimental.shard_map import shard_map
    import concourse.mybir as mybir
    import concourse.bass2jax as bass2jax

    bass2jax.install_neuronx_cc_hook()

    partition_name = (nc.partition_id_tensor.name
                      if nc.partition_id_tensor else None)
    in_names, out_names, out_avals, zero_outs = [], [], [], []
    for alloc in nc.m.functions[0].allocations:
        if not isinstance(alloc, mybir.MemoryLocationSet):
            continue
        name = alloc.memorylocations[0].name
        if alloc.kind == "ExternalInput":
            if name != partition_name:
                in_names.append(name)
        elif alloc.kind == "ExternalOutput":
            out_names.append(name)
            shape = tuple(alloc.tensor_shape)
            dtype = mybir.dt.np(alloc.dtype)
            out_avals.append(jax.core.ShapedArray(shape, dtype))
            zero_outs.append(np.zeros(shape, dtype))
    n_params = len(in_names)
    all_in_names = list(in_names) + list(out_names)
    if partition_name is not None:
        all_in_names.append(partition_name)

    def _body(*args):
        operands = list(args)
        if partition_name is not None:
            operands.append(bass2jax.partition_id_tensor())
        outs = bass2jax._bass_exec_p.bind(
            *operands,
            out_avals=tuple(out_avals),
            in_names=tuple(all_in_names),
            out_names=tuple(out_names),
            lowering_input_output_aliases=(),
            sim_require_finite=True,
            sim_require_nnan=True,
            nc=nc,
        )
        return tuple(outs)

    devices = jax.devices()[:NCORES]
    mesh = Mesh(np.asarray(devices), ("core",))
    spec = PartitionSpec("core")
    in_specs = (spec,) * (n_params + len(out_names))
    out_specs = (spec,) * len(out_names)
    donate = tuple(range(n_params, n_params + len(out_names)))
    fn = jax.jit(
        shard_map(_body, mesh=mesh, in_specs=in_specs, out_specs=out_specs,
                  check_rep=False),
        donate_argnums=donate, keep_unused=True)
    sharding = NamedSharding(mesh, spec)
    return fn, in_names, zero_outs, sharding


def _make_caller(nc, in_maps):
    import time
    import jax

    fn, in_names, zero_outs, sharding = _make_sharded_callable(nc)
    concat_in = [
        jax.device_put(
            np.concatenate([np.asarray(in_maps[c][n]) for c in range(NCORES)],
                           axis=0), sharding)
        for n in in_names
    ]

    def one_call():
        zeros = [
            jax.device_put(
                np.zeros((NCORES * z.shape[0], *z.shape[1:]), z.dtype),
                sharding)
            for z in zero_outs
        ]
        for z in zeros:
            z.block_until_ready()
        t0 = time.monotonic()
        outs = fn(*concat_in, *zeros)
        for o in outs:
            o.block_until_ready()
        return time.monotonic() - t0

    return one_call


def _timed_exec(nc, in_maps, iters):
    one_call = _make_caller(nc, in_maps)
    one_call()  # compile + warm
    times = [one_call() for _ in range(iters)]
    return float(np.median(times)), times


def measure_hw_time_ns(iters=15, reps=49):
    """Estimated on-device exec time via the slope method.

    T1 and T_reps calls are interleaved so ambient load drift affects both
    phases equally; reps=49 amplifies the per-rep signal 48x over the
    per-call wall jitter.  est = (min(tR) - min(t1)) / (reps - 1).
    """
    nc1 = _prog_cache.get("nc")
    in_maps = _prog_cache.get("in_maps")
    if nc1 is None or in_maps is None:
        raise RuntimeError("run kernel() first")
    key = f"ncR{reps}"
    if key not in _prog_cache:
        _prog_cache[key] = _build_program(reps=reps)
    ncR = _prog_cache[key]
    call1 = _make_caller(nc1, in_maps)
    callR = _make_caller(ncR, in_maps)
    call1()  # compile + warm
    callR()
    t1_all, tR_all = [], []
    for _ in range(iters):
        t1_all.append(call1())
        tR_all.append(callR())
    t1 = min(t1_all)
    tR = min(tR_all)
    est = (tR - t1) / (reps - 1)
    return (est * 1e9, t1 * 1e9, tR * 1e9,
            [t * 1e9 for t in t1_all], [t * 1e9 for t in tR_all])

